# Initial kernel scaffold
#
"""Trainium2 Bass kernel for EdgeSelectionRL (gnn_message_passing).

Reference math (per batch b):
    a = xa @ Wa.T            (C, H)
    c = xa @ Wb.T            (C, H)
    logit[i, j] = sum_h w2[h] * relu(a[i, h] + c[j, h] + b1[h]) + b2
    out = sigmoid(logit)     (C, C)

Sharding: pure data-parallel over batch B=8 -> one batch element per core.

Per-core pipeline (h lives on partitions, two 128-chunks):
  setup: aT[h,i] (f32 SBUF) and cT_pre[h,j]=c.T+b1 (bf16 SBUF + f32 PSUM)
  main:  for each of 128 i-pairs x 2 h-chunks, produce
         R = relu(cT_pre + aT[:,i]) as (128h x 512) bf16 tiles
         (VectorE tensor_scalar add+max from SBUF, ScalarE activation Relu
         from PSUM - split tuned so both engines finish together), then
         TensorE reduces against w2 (M=32 replicated columns, N=512)
         accumulating into PSUM rows at partition 32*grp.
  out:   per 8-pair sweep (2 PSUM banks x 4 col-groups) one ScalarE sigmoid
         over the psum region; partition-strided DMA picks the valid rows.
"""

import numpy as np

B, C, F, H = 8, 256, 128, 256
NCORES = 8
NPAIR = C // 2            # 128 i-pairs per core
PAIRS_PER_SWEEP = 8       # 2 banks x 4 col-groups
NSWEEP = NPAIR // PAIRS_PER_SWEEP  # 16
ACT_SHARE = 150           # of 512 producer instrs on ScalarE
SIG_DEFER_AT = 5          # emit sweep s-1's sigmoid after this pair of sweep s

_cached = {}


def _build():
    import concourse.bass as bass
    import concourse.bacc as bacc
    import concourse.mybir as mybir
    from concourse import tile

    fp32 = mybir.dt.float32
    bf16 = mybir.dt.bfloat16
    Alu = mybir.AluOpType
    Act = mybir.ActivationFunctionType

    nc = bacc.Bacc(None, target_bir_lowering=False)

    xat_d = nc.dram_tensor("xat", [F, C], fp32, kind="ExternalInput")
    w1t_d = nc.dram_tensor("w1t", [2 * F, H], fp32, kind="ExternalInput")
    bcv_d = nc.dram_tensor("bcv", [128, 3], fp32, kind="ExternalInput")
    w2p_d = nc.dram_tensor("w2p", [128, 64], bf16, kind="ExternalInput")
    out_d = nc.dram_tensor("out", [C, C], fp32, kind="ExternalOutput")

    with tile.TileContext(nc) as tc:
        with (
            tc.tile_pool(name="const", bufs=1) as const_pool,
            tc.tile_pool(name="rtiles", bufs=16) as r_pool,
            tc.tile_pool(name="sig", bufs=4) as sig_pool,
            tc.tile_pool(name="psum", bufs=3, space=bass.MemorySpace.PSUM) as ps_pool,
            tc.tile_pool(name="psumc", bufs=1, space=bass.MemorySpace.PSUM) as psc_pool,
        ):
            # ---- load inputs ----
            xat = const_pool.tile([F, C], fp32, tag="xat")
            w1t = const_pool.tile([128, 2 * H], fp32, tag="w1t")  # [p, m2*H+h] = W1T[m2*128+p, h]
            bcv = const_pool.tile([128, 3], fp32, tag="bcv")      # b1 chunk0, chunk1, b2
            w2p = const_pool.tile([128, 64], bf16, tag="w2p")
            nc.sync.dma_start(xat[:], xat_d[:])
            nc.sync.dma_start(w1t[:, 0:H], w1t_d[0:128, :])
            nc.sync.dma_start(w1t[:, H:2 * H], w1t_d[128:256, :])
            nc.sync.dma_start(bcv[:], bcv_d[:])
            nc.sync.dma_start(w2p[:], w2p_d[:])
            w1t0 = w1t[:, 0:H]
            w1t1 = w1t[:, H:2 * H]
            b1p = bcv[:, 0:2]
            b2v = bcv[:, 2:3]

            # ---- setup ----
            warm = const_pool.tile([128, 1], fp32, tag="warm")
            nc.scalar.activation(
                warm[:], nc.const_aps.aps[(fp32, 0.0)], Act.Sigmoid,
            )

            aT = [const_pool.tile([128, C], fp32, tag=f"aT{m}", name=f"aT{m}")
                  for m in range(2)]
            aTb = [const_pool.tile([128, C], fp32, tag=f"aTb{m}", name=f"aTb{m}")
                   for m in range(2)]
            cT = [const_pool.tile([128, C], bf16, tag=f"cT{m}", name=f"cT{m}")
                  for m in range(2)]
            cTp = [psc_pool.tile([128, C], fp32, tag=f"cTp{m}", name=f"cTp{m}")
                   for m in range(2)]
            for m in range(2):
                ps = ps_pool.tile([128, 1024], fp32, tag="ps")
                nc.tensor.matmul(
                    ps[:, 0:C], w1t0[:, m * 128:(m + 1) * 128], xat[:],
                    start=True, stop=True,
                )
                nc.scalar.copy(aT[m][:], ps[:, 0:C])
                nc.scalar.activation(
                    aTb[m][:], ps[:, 0:C], Act.Identity, bias=b1p[:, m:m + 1],
                )
                nc.tensor.matmul(
                    cTp[m][:], w1t1[:, m * 128:(m + 1) * 128], xat[:],
                    start=True, stop=True,
                )
                nc.scalar.activation(
                    cT[m][:], cTp[m][:], Act.Identity, bias=b1p[:, m:m + 1],
                )

            # ---- main loop ----
            def _emit_sig(s, ps):
                sig = sig_pool.tile([128, 1024], fp32, tag="sig", name=f"sig{s}")
                nc.scalar.activation(sig[:], ps[:], Act.Sigmoid, bias=b2v[:, 0:1])
                # valid rows: partition 32*grp, free bank*512+hh*256 ->
                # out row i = 16*s + 8*bank + 2*grp + hh
                srcap = sig[0:128:32, :].rearrange("g (b e) -> g b e", b=2)
                dstap = out_d.rearrange(
                    "(S b g two) j -> S g b (two j)", S=NSWEEP, b=2, g=4, two=2
                )[s]
                nc.sync.dma_start(dstap, srcap)

            def _emit_sig_bank(bk, ps):
                # final-sweep tail: per-bank sigmoid, rows 240+8*bk..247+8*bk
                sigb = sig_pool.tile([128, 512], fp32, tag="sig", name=f"sigb{bk}")
                nc.scalar.activation(sigb[:], ps[:, bk * 512:(bk + 1) * 512],
                                     Act.Sigmoid, bias=b2v[:, 0:1])
                dstb = out_d[240 + 8 * bk:248 + 8 * bk, :].rearrange(
                    "(g two) j -> g (two j)", g=4)
                nc.sync.dma_start(dstb, sigb[0:128:32, :])

            pending = None
            for s in range(NSWEEP):
                ps = ps_pool.tile([128, 1024], fp32, tag="ps")
                for t in range(PAIRS_PER_SWEEP):
                    q = s * PAIRS_PER_SWEEP + t   # pair; i = 2q, 2q+1
                    bank = t // 4
                    grp = t % 4
                    rts = [r_pool.tile([128, 512], bf16, tag="r", name=f"r{q}_{m}")
                           for m in range(2)]
                    if t == SIG_DEFER_AT and pending is not None:
                        _emit_sig(*pending)
                        pending = None
                    for m in range(2):
                        for hh in range(2):
                            idx = 4 * q + 2 * m + hh
                            is_act = (idx % 10) < 3 and (idx // 10) % 26 != 5
                            i = 2 * q + hh
                            dst = rts[m][:, hh * 256:(hh + 1) * 256]
                            if is_act:
                                nc.scalar.activation(
                                    dst, cTp[m][:], Act.Relu,
                                    bias=aTb[m][:, i:i + 1],
                                )
                            else:
                                nc.vector.tensor_scalar(
                                    dst, cT[m][:], aT[m][:, i:i + 1], 0.0,
                                    Alu.add, Alu.max,
                                )
                    po = ps[32 * grp:32 * grp + 32, bank * 512:(bank + 1) * 512]
                    nc.tensor.matmul(po, w2p[:, 0:32], rts[0][:],
                                     start=True, stop=False,
                                     tile_position=(0, 32 * grp))
                    nc.tensor.matmul(po, w2p[:, 32:64], rts[1][:],
                                     start=False, stop=True,
                                     tile_position=(0, 32 * grp))
                    if s == NSWEEP - 1 and t == 3:
                        _emit_sig_bank(0, ps)

                pending = (s, ps)
            _emit_sig_bank(1, pending[1])

    nc.compile()
    return nc


def _prep_in_maps(xa, W1, b1, w2, b2):
    import ml_dtypes

    xa = np.asarray(xa, dtype=np.float32)
    W1 = np.asarray(W1, dtype=np.float32)
    b1 = np.asarray(b1, dtype=np.float32).reshape(H)
    w2 = np.asarray(w2, dtype=np.float32).reshape(H)
    b2 = np.float32(np.asarray(b2).reshape(()))

    w1t = np.ascontiguousarray(W1.T)                      # (2F, H)
    bcv = np.empty((128, 3), dtype=np.float32)
    bcv[:, 0:2] = b1.reshape(2, 128).T
    bcv[:, 2] = b2
    w2p = np.repeat(
        np.ascontiguousarray(w2.reshape(2, 128).T)[:, :, None], 32, axis=2
    ).reshape(128, 64).astype(ml_dtypes.bfloat16)         # [p, m*32+r] = w2[m*128+p]
    in_maps = []
    for k in range(NCORES):
        in_maps.append({
            "xat": np.ascontiguousarray(xa[k].T),         # (F, C)
            "w1t": w1t,
            "bcv": bcv,
            "w2p": w2p,
        })
    return in_maps


def kernel(xa, W1, b1, w2, b2):
    from concourse import bass_utils

    if "nc" not in _cached:
        _cached["nc"] = _build()
    nc = _cached["nc"]

    in_maps = _prep_in_maps(xa, W1, b1, w2, b2)
    res = bass_utils.run_bass_kernel_spmd(nc, in_maps, core_ids=list(range(NCORES)))
    out = np.stack([np.asarray(r["out"], dtype=np.float32) for r in res.results])
    return out



# revision 1
# speedup vs baseline: 5.6883x; 5.6883x over previous
"""Trainium2 Bass kernel for EdgeSelectionRL (gnn_message_passing).

Reference math (per batch b):
    a = xa @ Wa.T            (C, H)
    c = xa @ Wb.T            (C, H)
    logit[i, j] = sum_h w2[h] * relu(a[i, h] + c[j, h] + b1[h]) + b2
    out = sigmoid(logit)     (C, C)

Sharding: pure data-parallel over batch B=8 -> one batch element per core.

Per-core pipeline (h lives on partitions, two 128-chunks):
  setup: aT[h,i] (f32 SBUF) and cT_pre[h,j]=c.T+b1 (bf16 SBUF + f32 PSUM)
  main:  for each of 128 i-pairs x 2 h-chunks, produce
         R = relu(cT_pre + aT[:,i]) as (128h x 512) bf16 tiles
         (VectorE tensor_scalar add+max from SBUF, ScalarE activation Relu
         from PSUM - split tuned so both engines finish together), then
         TensorE reduces against w2 (M=32 replicated columns, N=512)
         accumulating into PSUM rows at partition 32*grp.
  out:   per 8-pair sweep (2 PSUM banks x 4 col-groups) one ScalarE sigmoid
         over the psum region; partition-strided DMA picks the valid rows.
"""

import numpy as np

B, C, F, H = 8, 256, 128, 256
NCORES = 8
NPAIR = C // 2            # 128 i-pairs per core
PAIRS_PER_SWEEP = 8       # 2 banks x 4 col-groups
NSWEEP = NPAIR // PAIRS_PER_SWEEP  # 16
ACT_SHARE = 150           # of 512 producer instrs on ScalarE
SIG_DEFER_AT = 5          # emit sweep s-1's sigmoid after this pair of sweep s

_cached = {}


def _build():
    import concourse.bass as bass
    import concourse.bacc as bacc
    import concourse.mybir as mybir
    from concourse import tile

    fp32 = mybir.dt.float32
    bf16 = mybir.dt.bfloat16
    Alu = mybir.AluOpType
    Act = mybir.ActivationFunctionType

    nc = bacc.Bacc(None, target_bir_lowering=False)

    xat_d = nc.dram_tensor("xat", [F, C], fp32, kind="ExternalInput")
    w1t_d = nc.dram_tensor("w1t", [2 * F, H], fp32, kind="ExternalInput")
    bcv_d = nc.dram_tensor("bcv", [128, 3], fp32, kind="ExternalInput")
    w2p_d = nc.dram_tensor("w2p", [128, 64], bf16, kind="ExternalInput")
    out_d = nc.dram_tensor("out", [C, C], fp32, kind="ExternalOutput")

    with tile.TileContext(nc) as tc:
        with (
            tc.tile_pool(name="const", bufs=1) as const_pool,
            tc.tile_pool(name="rtiles", bufs=16) as r_pool,
            tc.tile_pool(name="sig", bufs=4) as sig_pool,
            tc.tile_pool(name="psum", bufs=3, space=bass.MemorySpace.PSUM) as ps_pool,
            tc.tile_pool(name="psumc", bufs=1, space=bass.MemorySpace.PSUM) as psc_pool,
        ):
            # ---- load inputs ----
            xat = const_pool.tile([F, C], fp32, tag="xat")
            w1t = const_pool.tile([128, 2 * H], fp32, tag="w1t")  # [p, m2*H+h] = W1T[m2*128+p, h]
            bcv = const_pool.tile([128, 3], fp32, tag="bcv")      # b1 chunk0, chunk1, b2
            w2p = const_pool.tile([128, 64], bf16, tag="w2p")
            nc.sync.dma_start(xat[:], xat_d[:])
            nc.sync.dma_start(w1t[:, 0:H], w1t_d[0:128, :])
            nc.sync.dma_start(w1t[:, H:2 * H], w1t_d[128:256, :])
            nc.sync.dma_start(bcv[:], bcv_d[:])
            nc.sync.dma_start(w2p[:], w2p_d[:])
            w1t0 = w1t[:, 0:H]
            w1t1 = w1t[:, H:2 * H]
            b1p = bcv[:, 0:2]
            b2v = bcv[:, 2:3]

            # ---- setup ----
            warm = const_pool.tile([128, 1], fp32, tag="warm")
            nc.scalar.activation(
                warm[:], nc.const_aps.aps[(fp32, 0.0)], Act.Sigmoid,
            )

            aT = [const_pool.tile([128, C], fp32, tag=f"aT{m}", name=f"aT{m}")
                  for m in range(2)]
            aTb = [const_pool.tile([128, C], fp32, tag=f"aTb{m}", name=f"aTb{m}")
                   for m in range(2)]
            cT = [const_pool.tile([128, C], bf16, tag=f"cT{m}", name=f"cT{m}")
                  for m in range(2)]
            cTp = [psc_pool.tile([128, C], fp32, tag=f"cTp{m}", name=f"cTp{m}")
                   for m in range(2)]
            for m in range(2):
                ps = ps_pool.tile([128, 1024], fp32, tag="ps")
                nc.tensor.matmul(
                    ps[:, 0:C], w1t0[:, m * 128:(m + 1) * 128], xat[:],
                    start=True, stop=True,
                )
                nc.scalar.copy(aT[m][:], ps[:, 0:C])
                nc.scalar.activation(
                    aTb[m][:], ps[:, 0:C], Act.Identity, bias=b1p[:, m:m + 1],
                )
                nc.tensor.matmul(
                    cTp[m][:], w1t1[:, m * 128:(m + 1) * 128], xat[:],
                    start=True, stop=True,
                )
                nc.scalar.activation(
                    cT[m][:], cTp[m][:], Act.Identity, bias=b1p[:, m:m + 1],
                )

            # ---- main loop ----
            def _emit_sig(s, ps):
                sig = sig_pool.tile([128, 1024], fp32, tag="sig", name=f"sig{s}")
                nc.scalar.activation(sig[:], ps[:], Act.Sigmoid, bias=b2v[:, 0:1])
                # valid rows: partition 32*grp, free bank*512+hh*256 ->
                # out row i = 16*s + 8*bank + 2*grp + hh
                srcap = sig[0:128:32, :].rearrange("g (b e) -> g b e", b=2)
                dstap = out_d.rearrange(
                    "(S b g two) j -> S g b (two j)", S=NSWEEP, b=2, g=4, two=2
                )[s]
                nc.sync.dma_start(dstap, srcap)

            def _emit_sig_bank(bk, ps):
                # final-sweep tail: per-bank sigmoid, rows 240+8*bk..247+8*bk
                sigb = sig_pool.tile([128, 512], fp32, tag="sig", name=f"sigb{bk}")
                nc.scalar.activation(sigb[:], ps[:, bk * 512:(bk + 1) * 512],
                                     Act.Sigmoid, bias=b2v[:, 0:1])
                dstb = out_d[240 + 8 * bk:248 + 8 * bk, :].rearrange(
                    "(g two) j -> g (two j)", g=4)
                nc.sync.dma_start(dstb, sigb[0:128:32, :])

            pending = None
            for s in range(NSWEEP):
                ps = ps_pool.tile([128, 1024], fp32, tag="ps")
                for t in range(PAIRS_PER_SWEEP):
                    q = s * PAIRS_PER_SWEEP + t   # pair; i = 2q, 2q+1
                    bank = t // 4
                    grp = t % 4
                    rts = [r_pool.tile([128, 512], bf16, tag="r", name=f"r{q}_{m}")
                           for m in range(2)]
                    if t == SIG_DEFER_AT and pending is not None:
                        _emit_sig(*pending)
                        pending = None
                    for m in range(2):
                        for hh in range(2):
                            idx = 4 * q + 2 * m + hh
                            is_act = (idx % 10) < 3 and (idx // 10) % 26 != 5
                            i = 2 * q + hh
                            dst = rts[m][:, hh * 256:(hh + 1) * 256]
                            if is_act:
                                nc.scalar.activation(
                                    dst, cTp[m][:], Act.Relu,
                                    bias=aTb[m][:, i:i + 1],
                                )
                            else:
                                nc.vector.tensor_scalar(
                                    dst, cT[m][:], aT[m][:, i:i + 1], 0.0,
                                    Alu.add, Alu.max,
                                )
                    po = ps[32 * grp:32 * grp + 32, bank * 512:(bank + 1) * 512]
                    nc.tensor.matmul(po, w2p[:, 0:32], rts[0][:],
                                     start=True, stop=False,
                                     tile_position=(0, 32 * grp))
                    nc.tensor.matmul(po, w2p[:, 32:64], rts[1][:],
                                     start=False, stop=True,
                                     tile_position=(0, 32 * grp))
                    if s == NSWEEP - 1 and t == 3:
                        _emit_sig_bank(0, ps)

                pending = (s, ps)
            _emit_sig_bank(1, pending[1])

    nc.compile()
    return nc


def _prep_in_maps(xa, W1, b1, w2, b2):
    import ml_dtypes

    xa = np.asarray(xa, dtype=np.float32)
    W1 = np.asarray(W1, dtype=np.float32)
    b1 = np.asarray(b1, dtype=np.float32).reshape(H)
    w2 = np.asarray(w2, dtype=np.float32).reshape(H)
    b2 = np.float32(np.asarray(b2).reshape(()))

    w1t = np.ascontiguousarray(W1.T)                      # (2F, H)
    bcv = np.empty((128, 3), dtype=np.float32)
    bcv[:, 0:2] = b1.reshape(2, 128).T
    bcv[:, 2] = b2
    w2p = np.repeat(
        np.ascontiguousarray(w2.reshape(2, 128).T)[:, :, None], 32, axis=2
    ).reshape(128, 64).astype(ml_dtypes.bfloat16)         # [p, m*32+r] = w2[m*128+p]
    in_maps = []
    for k in range(NCORES):
        in_maps.append({
            "xat": np.ascontiguousarray(xa[k].T),         # (F, C)
            "w1t": w1t,
            "bcv": bcv,
            "w2p": w2p,
        })
    return in_maps


def kernel(xa, W1, b1, w2, b2):
    from concourse import bass_utils

    if "nc" not in _cached:
        _cached["nc"] = _build()
    nc = _cached["nc"]

    in_maps = _prep_in_maps(xa, W1, b1, w2, b2)
    res = bass_utils.run_bass_kernel_spmd(nc, in_maps, core_ids=list(range(NCORES)))
    out = np.stack([np.asarray(r["out"], dtype=np.float32) for r in res.results])
    return out

